# revision 38
# baseline (speedup 1.0000x reference)
"""Trainium2 Bass kernel for DepthwiseXCorr (SiamRPN-style head).

Pipeline per batch sample:
  k = BN+ReLU(conv1x1(kernel, w_k))      [256, 7, 7]
  s = BN+ReLU(conv1x1(search, w_s))      [256, 31, 31]
  feat = depthwise_xcorr(s, k)           [256, 25, 25]
  y = BN+ReLU(conv1x1(feat, w_h1))
  out = conv1x1(y, w_h2) + bias          [20, 25, 25]

Sharding: pure data-parallel, batch 128 -> 16 per core across 8 cores.

Implementation notes:
- conv1x1 = matmul with channels on the contraction (partition) dim.
- depthwise xcorr: the 49 kernel taps (u, v) are split across engines.
  33 taps run on the PE as diag(k[:, u, v]) @ s[:, i+u, j+v] matmuls
  accumulating in PSUM (a depthwise tap can use at most 128 of the
  128x128 array's rows, so the PE alone would need ~410us); the diagonal
  weight tiles are built by scaling an identity matrix with a
  per-partition scalar (12 on VectorE at 4x bf16 mode, 21 on ScalarE,
  which has slack).  The other 16 taps run on VectorE as 4x-mode
  tensor_scalar mults plus 2x-mode tensor_tensor adds into a bf16
  accumulator that is folded into the PSUM group by one extra
  identity-weight matmul per phase.  Shifted search windows are plain
  strided APs into the search feature tile - no data movement.
- All matmuls in bf16 (PE full column rate), accumulation in fp32 PSUM,
  BN+ReLU epilogues on ScalarE with fp32 math, fp32 output.
- Cost-model span ~353 us; measured marginal HW time ~380-430 us
  (all three compute engines ~90% busy).
"""

import sys

if "/opt/trn_rl_repo" not in sys.path:
    sys.path.insert(0, "/opt/trn_rl_repo")

import ml_dtypes
import numpy as np

B, CIN, HID, COUT = 128, 256, 256, 20
NCORES = 8
NB = B // NCORES          # batches per core
HS = 31                   # search spatial
HK = 7                    # kernel spatial
HO = HS - HK + 1          # 25, xcorr output spatial
EPS = 1e-5
GB = 4                    # batch group size for the search-branch pipeline
NCH = 2                   # channel chunks of 128
P_TAPS = 33               # xcorr taps on PE (diag matmuls)
Q_TAPS = 0                # taps whose mult runs on ScalarE (add on VectorE)
D_TAPS = 49 - P_TAPS - Q_TAPS  # taps fully on VectorE (mult + add pairs)
DIAG_DVE = 11             # diag builds on DVE; the rest go to ScalarE
BF16 = ml_dtypes.bfloat16

_CACHE = {}


def _build_nc(repeat=1):
    import concourse.bacc as bacc
    import concourse.tile as tile
    from concourse import mybir

    f32 = mybir.dt.float32
    bf16 = mybir.dt.bfloat16

    nc = bacc.Bacc()

    xk = nc.dram_tensor("xk", [NB, CIN, HK, HK], bf16, kind="ExternalInput")
    xs = nc.dram_tensor("xs", [NB, CIN, HS, HS], bf16, kind="ExternalInput")
    wkT = nc.dram_tensor("wkT", [CIN, HID], bf16, kind="ExternalInput")
    wsT = nc.dram_tensor("wsT", [CIN, HID], bf16, kind="ExternalInput")
    wh1T = nc.dram_tensor("wh1T", [HID, HID], bf16, kind="ExternalInput")
    wh2T = nc.dram_tensor("wh2T", [HID, COUT], bf16, kind="ExternalInput")
    sck = nc.dram_tensor("sck", [HID, 1], f32, kind="ExternalInput")
    shk = nc.dram_tensor("shk", [HID, 1], f32, kind="ExternalInput")
    scs = nc.dram_tensor("scs", [HID, 1], f32, kind="ExternalInput")
    shs = nc.dram_tensor("shs", [HID, 1], f32, kind="ExternalInput")
    sch = nc.dram_tensor("sch", [HID, 1], f32, kind="ExternalInput")
    shh = nc.dram_tensor("shh", [HID, 1], f32, kind="ExternalInput")
    bh2 = nc.dram_tensor("bh2", [COUT, 1], f32, kind="ExternalInput")
    ident = nc.dram_tensor("ident", [128, 128], bf16, kind="ExternalInput")
    out = nc.dram_tensor("out", [NB, COUT, HO, HO], f32, kind="ExternalOutput")

    relu = mybir.ActivationFunctionType.Relu
    idfn = mybir.ActivationFunctionType.Identity
    copyfn = mybir.ActivationFunctionType.Copy

    with tile.TileContext(nc) as tc:
        with (
            tc.tile_pool(name="const", bufs=1) as cpool,
            tc.tile_pool(name="act", bufs=1) as apool,
            tc.tile_pool(name="stream", bufs=2) as spool,
            tc.tile_pool(name="diagp", bufs=2 * P_TAPS + 14) as dpool,
            tc.tile_pool(name="psum", bufs=1, space="PSUM") as ppool,
        ):
            # ---- constants -------------------------------------------------
            # critical-path loads first: the kernel branch gates the xcorr
            # diag builds, so wk/sck/shk/ident/xk go ahead of everything.
            wk_t, ws_t, wh1_t, wh2_t = [], [], [], []
            sck_t, shk_t, scs_t, shs_t, sch_t, shh_t = [], [], [], [], [], []

            def _vec(vec_d, lst, nm, kc, sl):
                v = cpool.tile([128, 1], f32, name=f"{nm}_{kc}")
                nc.sync.dma_start(v[:], vec_d[sl, :])
                lst.append(v)

            for kc in range(NCH):
                sl = slice(kc * 128, (kc + 1) * 128)
                w1 = cpool.tile([128, HID], bf16, name=f"wk_{kc}")
                nc.sync.dma_start(w1[:], wkT[sl, :])
                wk_t.append(w1)
                _vec(sck, sck_t, "sck", kc, sl)
                _vec(shk, shk_t, "shk", kc, sl)
            id_t = cpool.tile([128, 128], bf16, name="id_t")
            nc.sync.dma_start(id_t[:], ident[:])
            xk_ap = xk[:].rearrange("b c h w -> c b (h w)")
            xk_t = []
            for kc in range(NCH):
                t = apool.tile([128, NB, HK * HK], bf16, name=f"xk_t{kc}")
                nc.gpsimd.dma_start(t[:], xk_ap[kc * 128:(kc + 1) * 128])
                xk_t.append(t)
            for kc in range(NCH):
                sl = slice(kc * 128, (kc + 1) * 128)
                w2 = cpool.tile([128, HID], bf16, name=f"ws_{kc}")
                nc.sync.dma_start(w2[:], wsT[sl, :])
                ws_t.append(w2)
                _vec(scs, scs_t, "scs", kc, sl)
                _vec(shs, shs_t, "shs", kc, sl)
            for kc in range(NCH):
                sl = slice(kc * 128, (kc + 1) * 128)
                w3 = cpool.tile([128, HID], bf16, name=f"wh1_{kc}")
                nc.sync.dma_start(w3[:], wh1T[sl, :])
                wh1_t.append(w3)
                w4 = cpool.tile([128, COUT], bf16, name=f"wh2_{kc}")
                nc.sync.dma_start(w4[:], wh2T[sl, :])
                wh2_t.append(w4)
                _vec(sch, sch_t, "sch", kc, sl)
                _vec(shh, shh_t, "shh", kc, sl)
            bh2_t = cpool.tile([COUT, 1], f32, name="bh2_t")
            nc.sync.dma_start(bh2_t[:], bh2[:])

            # ---- kernel branch conv (all NB batches at once) ---------------
            # k_feat[mc]: [128, NB, 49] fp32 (source of per-partition scalars)
            k_feat = []
            for mc in range(NCH):
                kf = apool.tile([128, NB, HK * HK], f32, name=f"k_feat{mc}")
                for half in range(2):
                    bs = slice(half * (NB // 2), (half + 1) * (NB // 2))
                    ps = ppool.tile([128, NB // 2, HK * HK], f32,
                                    name="ps_cs", tag="csA", bufs=1)
                    for kc in range(NCH):
                        nc.tensor.matmul(
                            ps[:],
                            wk_t[kc][:, mc * 128:(mc + 1) * 128],
                            xk_t[kc][:, bs, :],
                            start=(kc == 0), stop=(kc == NCH - 1),
                        )
                    nc.scalar.activation(kf[:, bs, :], ps[:], relu,
                                         bias=shk_t[mc][:], scale=sck_t[mc][:])
                k_feat.append(kf)

            for _rep in range(repeat):
              # ---- main loop over batch groups -------------------------------
              # xcorr iterations are (bl, mc) pairs
              NGRP = NB // GB
              iters = [(g, bl, mc) for g in range(NGRP)
                       for bl in range(GB) for mc in range(NCH)]
              pe_taps = {it: P_TAPS for it in iters}

              def emit_diags_dve(it):
                  g, bl, mc = it
                  b_abs = g * GB + bl
                  lst = []
                  for uv in range(DIAG_DVE):
                      dg = dpool.tile([128, 128], bf16, name="dg", tag="diag")
                      nc.vector.tensor_scalar_mul(
                          dg[:], id_t[:], k_feat[mc][:, b_abs, uv:uv + 1])
                      lst.append(dg)
                  return lst

              def emit_diags_act(it):
                  g, bl, mc = it
                  b_abs = g * GB + bl
                  lst = []
                  for uv in range(DIAG_DVE, pe_taps[it]):
                      dg = dpool.tile([128, 128], bf16, name="dg", tag="diag")
                      nc.scalar.mul(dg[:], id_t[:],
                                    k_feat[mc][:, b_abs, uv:uv + 1])
                      lst.append(dg)
                  return lst

              xs_ap = xs[:].rearrange("b c h w -> c b (h w)")
              feat = {}
              ys = {}

              def emit_group_conv(g):
                  """xs load + search conv + BN/ReLU for one batch group."""
                  gsl = slice(g * GB, (g + 1) * GB)
                  xs_t = []
                  for kc in range(NCH):
                      t = spool.tile([128, GB, HS * HS], bf16,
                                     name=f"xs_t{kc}", tag=f"xs{kc}")
                      nc.gpsimd.dma_start(t[:],
                                          xs_ap[kc * 128:(kc + 1) * 128, gsl])
                      xs_t.append(t)
                  s_feat = []
                  for mc in range(NCH):
                      sf = spool.tile([128, GB, HS, HS], bf16,
                                      name=f"s_feat{mc}", tag=f"sf{mc}")
                      for bl in range(GB):
                          for ph, (r0, r1) in enumerate(((0, 16), (16, 31))):
                              ps = ppool.tile([128, r1 - r0, HS], f32,
                                              name="ps_cs2",
                                              tag=("csA" if ph == 0 else "csB"),
                                              bufs=1)
                              for kc in range(NCH):
                                  nc.tensor.matmul(
                                      ps[:],
                                      ws_t[kc][:, mc * 128:(mc + 1) * 128],
                                      xs_t[kc][:, bl, r0 * HS:r1 * HS],
                                      start=(kc == 0), stop=(kc == NCH - 1),
                                  )
                              nc.scalar.activation(sf[:, bl, r0:r1, :], ps[:],
                                                   relu, bias=shs_t[mc][:],
                                                   scale=scs_t[mc][:])
                      s_feat.append(sf)
                  return s_feat

              s_feat = emit_group_conv(0)
              for g in range(NGRP):
                  s_feat_next = None
                  for bl in range(GB):
                      if bl == GB - 2 and g + 1 < NGRP:
                          # hoist the next group's conv ahead of this
                          # group's last batch so ScalarE/DVE are not
                          # starved at the group boundary
                          s_feat_next = emit_group_conv(g + 1)
                      b_abs = g * GB + bl
                      for mc in range(NCH):
                          it = (g, bl, mc)
                          kcol = lambda uv: k_feat[mc][:, b_abs, uv:uv + 1]
                          swin = lambda uv, r0, r1: s_feat[mc][
                              :, bl, uv // 7 + r0:uv // 7 + r1,
                              uv % 7:uv % 7 + HO]
                          diags = emit_diags_dve(it) + emit_diags_act(it)
                          # DVE taps -> bf16 accumulator (4x-mode mult into a
                          # temp + 2x-mode add; cheaper than the 1x fused STT)
                          n_pe = pe_taps[it]
                          n_dve = 49 - n_pe
                          mm_extra = []
                          if n_dve:
                              accd = apool.tile([128, HO, HO], bf16,
                                                name="accd", tag=f"accd{mc}",
                                                bufs=3)
                              for i in range(n_dve):
                                  uv = n_pe + i
                                  if i == 0:
                                      nc.vector.tensor_scalar_mul(
                                          accd[:], swin(uv, 0, HO), kcol(uv))
                                  else:
                                      tmp = apool.tile([128, HO, HO], bf16,
                                                       name="tmp", tag="tmp",
                                                       bufs=4)
                                      if i > n_dve - 1 - Q_TAPS:
                                          nc.scalar.mul(
                                              tmp[:], swin(uv, 0, HO),
                                              kcol(uv))
                                      else:
                                          nc.vector.tensor_scalar_mul(
                                              tmp[:], swin(uv, 0, HO),
                                              kcol(uv))
                                      nc.vector.tensor_tensor(
                                          accd[:], accd[:], tmp[:],
                                          mybir.AluOpType.add)
                              mm_extra.append(accd)
                          ft = apool.tile([128, HO * HO], bf16,
                                          name=f"feat{mc}", tag=f"f{mc}", bufs=2)
                          # phase A: output rows 0:20 (N=500)
                          psA = ppool.tile([128, 20 * HO], f32,
                                           name="ps_xc", tag="xc", bufs=3)
                          for uv in range(n_pe):
                              nc.tensor.matmul(
                                  psA[:], diags[uv][:], swin(uv, 0, 20),
                                  start=(uv == 0), stop=False,
                              )
                          for j, acc in enumerate(mm_extra):
                              nc.tensor.matmul(
                                  psA[:], id_t[:], acc[:, 0:20, :],
                                  start=False, stop=(j == len(mm_extra) - 1),
                              )
                          nc.scalar.activation(ft[:, 0:500], psA[:], copyfn)
                          # phase B: output rows 20:25 (N=125)
                          psB = ppool.tile([128, 5 * HO], f32,
                                           name="ps_xcB", tag="xc", bufs=3)
                          for uv in range(n_pe):
                              nc.tensor.matmul(
                                  psB[:], diags[uv][:], swin(uv, 20, 25),
                                  start=(uv == 0), stop=False,
                              )
                          for j, acc in enumerate(mm_extra):
                              nc.tensor.matmul(
                                  psB[:], id_t[:], acc[:, 20:25, :],
                                  start=False, stop=(j == len(mm_extra) - 1),
                              )
                          nc.scalar.activation(ft[:, 500:625], psB[:], copyfn)
                          feat[mc] = ft

                          if mc < NCH - 1:
                              continue

                          # head conv1 + BN/ReLU
                          for mq in range(NCH):
                              yt = apool.tile([128, HO * HO], bf16,
                                              name=f"y{mq}", tag=f"y{mq}",
                                              bufs=2)
                              for ph, (c0, c1) in enumerate(((0, 500),
                                                            (500, 625))):
                                  ps = ppool.tile([128, c1 - c0], f32,
                                                  name="ps_hd",
                                                  tag=("hdA" if ph == 0
                                                       else "hdB"),
                                                  bufs=(2 if ph == 0 else 1))
                                  for kc in range(NCH):
                                      nc.tensor.matmul(
                                          ps[:],
                                          wh1_t[kc][:, mq * 128:(mq + 1) * 128],
                                          feat[kc][:, c0:c1],
                                          start=(kc == 0), stop=(kc == NCH - 1),
                                      )
                                  nc.scalar.activation(yt[:, c0:c1], ps[:],
                                                       relu, bias=shh_t[mq][:],
                                                       scale=sch_t[mq][:])
                              ys[mq] = yt

                          # head conv2 + bias
                          ot = apool.tile([COUT, HO * HO], f32,
                                          name="ot", tag="ot", bufs=2)
                          for ph, (c0, c1) in enumerate(((0, 500), (500, 625))):
                              ps = ppool.tile([COUT, c1 - c0], f32,
                                              name="ps_o",
                                              tag=("hdA" if ph == 0 else "hdB"),
                                              bufs=(2 if ph == 0 else 1))
                              for kc in range(NCH):
                                  nc.tensor.matmul(
                                      ps[:],
                                      wh2_t[kc][:],
                                      ys[kc][:, c0:c1],
                                      start=(kc == 0), stop=(kc == NCH - 1),
                                  )
                              nc.scalar.activation(ot[:, c0:c1], ps[:], idfn,
                                                   bias=bh2_t[:], scale=1.0)
                          nc.sync.dma_start(
                              out[b_abs][:].rearrange("o h w -> o (h w)"), ot[:])
                  if s_feat_next is not None:
                      s_feat = s_feat_next

    nc.compile()
    return nc


def _get_nc():
    if "nc" not in _CACHE:
        _CACHE["nc"] = _build_nc()
    return _CACHE["nc"]


def kernel(kernel, search, w_k, g_k, b_k, m_k, v_k, w_s, g_s, b_s, m_s, v_s,
           w_h1, g_h, b_h, m_h, v_h, w_h2, bias_h2):
    from concourse.bass_utils import run_bass_kernel_spmd

    def fold(g, b, m, v):
        sc = (g / np.sqrt(v + EPS)).astype(np.float32)
        sh = (b - m * sc).astype(np.float32)
        return sc.reshape(-1, 1), sh.reshape(-1, 1)

    kernel, search, w_k, w_s, w_h1, w_h2, bias_h2 = [
        np.asarray(a) for a in
        (kernel, search, w_k, w_s, w_h1, w_h2, bias_h2)]
    g_k, b_k, m_k, v_k = map(np.asarray, (g_k, b_k, m_k, v_k))
    g_s, b_s, m_s, v_s = map(np.asarray, (g_s, b_s, m_s, v_s))
    g_h, b_h, m_h, v_h = map(np.asarray, (g_h, b_h, m_h, v_h))

    sck, shk = fold(g_k, b_k, m_k, v_k)
    scs, shs = fold(g_s, b_s, m_s, v_s)
    sch, shh = fold(g_h, b_h, m_h, v_h)

    common = {
        "wkT": np.ascontiguousarray(w_k.T).astype(BF16),
        "wsT": np.ascontiguousarray(w_s.T).astype(BF16),
        "wh1T": np.ascontiguousarray(w_h1.T).astype(BF16),
        "wh2T": np.ascontiguousarray(w_h2.T).astype(BF16),
        "sck": sck, "shk": shk, "scs": scs, "shs": shs,
        "sch": sch, "shh": shh,
        "bh2": bias_h2.astype(np.float32).reshape(-1, 1),
        "ident": np.eye(128, dtype=BF16),
    }
    xk_all = kernel.astype(BF16)
    xs_all = search.astype(BF16)

    in_maps = []
    for i in range(NCORES):
        bs = slice(i * NB, (i + 1) * NB)
        m = dict(common)
        m["xk"] = np.ascontiguousarray(xk_all[bs])
        m["xs"] = np.ascontiguousarray(xs_all[bs])
        in_maps.append(m)

    nc = _get_nc()
    res = run_bass_kernel_spmd(nc, in_maps, core_ids=list(range(NCORES)))
    return np.concatenate([res.results[i]["out"] for i in range(NCORES)],
                          axis=0)



# revision 42
# speedup vs baseline: 1.0444x; 1.0444x over previous
"""Trainium2 Bass kernel for DepthwiseXCorr (SiamRPN-style head).

Pipeline per batch sample:
  k = BN+ReLU(conv1x1(kernel, w_k))      [256, 7, 7]
  s = BN+ReLU(conv1x1(search, w_s))      [256, 31, 31]
  feat = depthwise_xcorr(s, k)           [256, 25, 25]
  y = BN+ReLU(conv1x1(feat, w_h1))
  out = conv1x1(y, w_h2) + bias          [20, 25, 25]

Sharding: pure data-parallel, batch 128 -> 16 per core across 8 cores.

Implementation notes:
- conv1x1 = matmul with channels on the contraction (partition) dim.
- depthwise xcorr: the 49 kernel taps (u, v) are split across engines.
  33 taps run on the PE as diag(k[:, u, v]) @ s[:, i+u, j+v] matmuls
  accumulating in PSUM (a depthwise tap can use at most 128 of the
  128x128 array's rows, so the PE alone would need ~410us); the diagonal
  weight tiles are built by scaling an identity matrix with a
  per-partition scalar (12 on VectorE at 4x bf16 mode, 21 on ScalarE,
  which has slack).  The other 16 taps run on VectorE as 4x-mode
  tensor_scalar mults plus 2x-mode tensor_tensor adds into a bf16
  accumulator that is folded into the PSUM group by one extra
  identity-weight matmul per phase.  Shifted search windows are plain
  strided APs into the search feature tile - no data movement.
- All matmuls in bf16 (PE full column rate), accumulation in fp32 PSUM,
  BN+ReLU epilogues on ScalarE with fp32 math, fp32 output.
- Cost-model span ~353 us; measured marginal HW time ~380-430 us
  (all three compute engines ~90% busy).
"""

import sys

if "/opt/trn_rl_repo" not in sys.path:
    sys.path.insert(0, "/opt/trn_rl_repo")

import ml_dtypes
import numpy as np

B, CIN, HID, COUT = 128, 256, 256, 20
NCORES = 8
NB = B // NCORES          # batches per core
HS = 31                   # search spatial
HK = 7                    # kernel spatial
HO = HS - HK + 1          # 25, xcorr output spatial
EPS = 1e-5
GB = 4                    # batch group size for the search-branch pipeline
NCH = 2                   # channel chunks of 128
P_TAPS = 33               # xcorr taps on PE (diag matmuls)
Q_TAPS = 0                # taps whose mult runs on ScalarE (add on VectorE)
D_TAPS = 49 - P_TAPS - Q_TAPS  # taps fully on VectorE (mult + add pairs)
DIAG_DVE = 11             # diag builds on DVE; the rest go to ScalarE
BF16 = ml_dtypes.bfloat16

_CACHE = {}


def _build_nc(repeat=1):
    import concourse.bacc as bacc
    import concourse.tile as tile
    from concourse import mybir

    f32 = mybir.dt.float32
    bf16 = mybir.dt.bfloat16

    nc = bacc.Bacc()

    xk = nc.dram_tensor("xk", [NB, CIN, HK, HK], bf16, kind="ExternalInput")
    xs = nc.dram_tensor("xs", [NB, CIN, HS, HS], bf16, kind="ExternalInput")
    wkT = nc.dram_tensor("wkT", [CIN, HID], bf16, kind="ExternalInput")
    wsT = nc.dram_tensor("wsT", [CIN, HID], bf16, kind="ExternalInput")
    wh1T = nc.dram_tensor("wh1T", [HID, HID], bf16, kind="ExternalInput")
    wh2T = nc.dram_tensor("wh2T", [HID, COUT], bf16, kind="ExternalInput")
    sck = nc.dram_tensor("sck", [HID, 1], f32, kind="ExternalInput")
    shk = nc.dram_tensor("shk", [HID, 1], f32, kind="ExternalInput")
    scs = nc.dram_tensor("scs", [HID, 1], f32, kind="ExternalInput")
    shs = nc.dram_tensor("shs", [HID, 1], f32, kind="ExternalInput")
    sch = nc.dram_tensor("sch", [HID, 1], f32, kind="ExternalInput")
    shh = nc.dram_tensor("shh", [HID, 1], f32, kind="ExternalInput")
    bh2 = nc.dram_tensor("bh2", [COUT, 1], f32, kind="ExternalInput")
    ident = nc.dram_tensor("ident", [128, 128], bf16, kind="ExternalInput")
    out = nc.dram_tensor("out", [NB, COUT, HO, HO], f32, kind="ExternalOutput")

    relu = mybir.ActivationFunctionType.Relu
    idfn = mybir.ActivationFunctionType.Identity
    copyfn = mybir.ActivationFunctionType.Copy

    with tile.TileContext(nc) as tc:
        with (
            tc.tile_pool(name="const", bufs=1) as cpool,
            tc.tile_pool(name="act", bufs=1) as apool,
            tc.tile_pool(name="stream", bufs=2) as spool,
            tc.tile_pool(name="diagp", bufs=3 * P_TAPS) as dpool,
            tc.tile_pool(name="psum", bufs=1, space="PSUM") as ppool,
        ):
            # ---- constants -------------------------------------------------
            # critical-path loads first: the kernel branch gates the xcorr
            # diag builds, so wk/sck/shk/ident/xk go ahead of everything.
            wk_t, ws_t, wh1_t, wh2_t = [], [], [], []
            sck_t, shk_t, scs_t, shs_t, sch_t, shh_t = [], [], [], [], [], []

            def _vec(vec_d, lst, nm, kc, sl):
                v = cpool.tile([128, 1], f32, name=f"{nm}_{kc}")
                nc.sync.dma_start(v[:], vec_d[sl, :])
                lst.append(v)

            for kc in range(NCH):
                sl = slice(kc * 128, (kc + 1) * 128)
                w1 = cpool.tile([128, HID], bf16, name=f"wk_{kc}")
                nc.sync.dma_start(w1[:], wkT[sl, :])
                wk_t.append(w1)
                _vec(sck, sck_t, "sck", kc, sl)
                _vec(shk, shk_t, "shk", kc, sl)
            id_t = cpool.tile([128, 128], bf16, name="id_t")
            nc.sync.dma_start(id_t[:], ident[:])
            xk_ap = xk[:].rearrange("b c h w -> c b (h w)")
            xk_t = []
            for kc in range(NCH):
                t = apool.tile([128, NB, HK * HK], bf16, name=f"xk_t{kc}")
                nc.gpsimd.dma_start(t[:], xk_ap[kc * 128:(kc + 1) * 128])
                xk_t.append(t)
            for kc in range(NCH):
                sl = slice(kc * 128, (kc + 1) * 128)
                w2 = cpool.tile([128, HID], bf16, name=f"ws_{kc}")
                nc.sync.dma_start(w2[:], wsT[sl, :])
                ws_t.append(w2)
                _vec(scs, scs_t, "scs", kc, sl)
                _vec(shs, shs_t, "shs", kc, sl)
            for kc in range(NCH):
                sl = slice(kc * 128, (kc + 1) * 128)
                w3 = cpool.tile([128, HID], bf16, name=f"wh1_{kc}")
                nc.sync.dma_start(w3[:], wh1T[sl, :])
                wh1_t.append(w3)
                w4 = cpool.tile([128, COUT], bf16, name=f"wh2_{kc}")
                nc.sync.dma_start(w4[:], wh2T[sl, :])
                wh2_t.append(w4)
                _vec(sch, sch_t, "sch", kc, sl)
                _vec(shh, shh_t, "shh", kc, sl)
            bh2_t = cpool.tile([COUT, 1], f32, name="bh2_t")
            nc.sync.dma_start(bh2_t[:], bh2[:])

            # ---- kernel branch conv (all NB batches at once) ---------------
            # k_feat[mc]: [128, NB, 49] fp32 (source of per-partition scalars)
            k_feat = []
            for mc in range(NCH):
                kf = apool.tile([128, NB, HK * HK], f32, name=f"k_feat{mc}")
                for half in range(2):
                    bs = slice(half * (NB // 2), (half + 1) * (NB // 2))
                    ps = ppool.tile([128, NB // 2, HK * HK], f32,
                                    name="ps_cs", tag="csA", bufs=1)
                    for kc in range(NCH):
                        nc.tensor.matmul(
                            ps[:],
                            wk_t[kc][:, mc * 128:(mc + 1) * 128],
                            xk_t[kc][:, bs, :],
                            start=(kc == 0), stop=(kc == NCH - 1),
                        )
                    nc.scalar.activation(kf[:, bs, :], ps[:], relu,
                                         bias=shk_t[mc][:], scale=sck_t[mc][:])
                k_feat.append(kf)

            for _rep in range(repeat):
              # ---- main loop over batch groups -------------------------------
              # xcorr iterations are (bl, mc) pairs
              NGRP = NB // GB
              iters = [(g, bl, mc) for g in range(NGRP)
                       for bl in range(GB) for mc in range(NCH)]
              pe_taps = {it: P_TAPS for it in iters}

              def emit_diags_dve(it):
                  g, bl, mc = it
                  b_abs = g * GB + bl
                  lst = []
                  for uv in range(DIAG_DVE):
                      dg = dpool.tile([128, 128], bf16, name="dg", tag="diag")
                      nc.vector.tensor_scalar_mul(
                          dg[:], id_t[:], k_feat[mc][:, b_abs, uv:uv + 1])
                      lst.append(dg)
                  return lst

              def emit_diags_act(it):
                  g, bl, mc = it
                  b_abs = g * GB + bl
                  lst = []
                  for uv in range(DIAG_DVE, pe_taps[it]):
                      dg = dpool.tile([128, 128], bf16, name="dg", tag="diag")
                      nc.scalar.mul(dg[:], id_t[:],
                                    k_feat[mc][:, b_abs, uv:uv + 1])
                      lst.append(dg)
                  return lst

              xs_ap = xs[:].rearrange("b c h w -> c b (h w)")
              feat = {}
              ys = {}

              def emit_group_conv(g):
                  """xs load + search conv + BN/ReLU for one batch group."""
                  gsl = slice(g * GB, (g + 1) * GB)
                  xs_t = []
                  for kc in range(NCH):
                      t = spool.tile([128, GB, HS * HS], bf16,
                                     name=f"xs_t{kc}", tag=f"xs{kc}")
                      nc.gpsimd.dma_start(t[:],
                                          xs_ap[kc * 128:(kc + 1) * 128, gsl])
                      xs_t.append(t)
                  s_feat = []
                  for mc in range(NCH):
                      sf = spool.tile([128, GB, HS, HS], bf16,
                                      name=f"s_feat{mc}", tag=f"sf{mc}")
                      for bl in range(GB):
                          for ph, (r0, r1) in enumerate(((0, 16), (16, 31))):
                              ps = ppool.tile([128, r1 - r0, HS], f32,
                                              name="ps_cs2",
                                              tag=("csA" if ph == 0 else "csB"),
                                              bufs=1)
                              for kc in range(NCH):
                                  nc.tensor.matmul(
                                      ps[:],
                                      ws_t[kc][:, mc * 128:(mc + 1) * 128],
                                      xs_t[kc][:, bl, r0 * HS:r1 * HS],
                                      start=(kc == 0), stop=(kc == NCH - 1),
                                  )
                              nc.scalar.activation(sf[:, bl, r0:r1, :], ps[:],
                                                   relu, bias=shs_t[mc][:],
                                                   scale=scs_t[mc][:])
                      s_feat.append(sf)
                  return s_feat

              s_feat = emit_group_conv(0)
              for g in range(NGRP):
                  s_feat_next = None
                  for bl in range(GB):
                      if bl == GB - 2 and g + 1 < NGRP:
                          # hoist the next group's conv ahead of this
                          # group's last batch so ScalarE/DVE are not
                          # starved at the group boundary
                          s_feat_next = emit_group_conv(g + 1)
                      b_abs = g * GB + bl
                      for mc in range(NCH):
                          it = (g, bl, mc)
                          kcol = lambda uv: k_feat[mc][:, b_abs, uv:uv + 1]
                          swin = lambda uv, r0, r1: s_feat[mc][
                              :, bl, uv // 7 + r0:uv // 7 + r1,
                              uv % 7:uv % 7 + HO]
                          diags = emit_diags_dve(it) + emit_diags_act(it)
                          # DVE taps -> bf16 accumulator (4x-mode mult into a
                          # temp + 2x-mode add; cheaper than the 1x fused STT)
                          n_pe = pe_taps[it]
                          n_dve = 49 - n_pe
                          mm_extra = []
                          if n_dve:
                              accd = apool.tile([128, HO, HO], bf16,
                                                name="accd", tag=f"accd{mc}",
                                                bufs=4)
                              for i in range(n_dve):
                                  uv = n_pe + i
                                  if i == 0:
                                      nc.vector.tensor_scalar_mul(
                                          accd[:], swin(uv, 0, HO), kcol(uv))
                                  else:
                                      tmp = apool.tile([128, HO, HO], bf16,
                                                       name="tmp", tag="tmp",
                                                       bufs=6)
                                      if i > n_dve - 1 - Q_TAPS:
                                          nc.scalar.mul(
                                              tmp[:], swin(uv, 0, HO),
                                              kcol(uv))
                                      else:
                                          nc.vector.tensor_scalar_mul(
                                              tmp[:], swin(uv, 0, HO),
                                              kcol(uv))
                                      nc.vector.tensor_tensor(
                                          accd[:], accd[:], tmp[:],
                                          mybir.AluOpType.add)
                              mm_extra.append(accd)
                          ft = apool.tile([128, HO * HO], bf16,
                                          name=f"feat{mc}", tag=f"f{mc}", bufs=3)
                          # phase A: output rows 0:20 (N=500)
                          psA = ppool.tile([128, 20 * HO], f32,
                                           name="ps_xc", tag="xc", bufs=3)
                          for uv in range(n_pe):
                              nc.tensor.matmul(
                                  psA[:], diags[uv][:], swin(uv, 0, 20),
                                  start=(uv == 0), stop=False,
                              )
                          for j, acc in enumerate(mm_extra):
                              nc.tensor.matmul(
                                  psA[:], id_t[:], acc[:, 0:20, :],
                                  start=False, stop=(j == len(mm_extra) - 1),
                              )
                          nc.scalar.activation(ft[:, 0:500], psA[:], copyfn)
                          # phase B: output rows 20:25 (N=125)
                          psB = ppool.tile([128, 5 * HO], f32,
                                           name="ps_xcB", tag="xc", bufs=3)
                          for uv in range(n_pe):
                              nc.tensor.matmul(
                                  psB[:], diags[uv][:], swin(uv, 20, 25),
                                  start=(uv == 0), stop=False,
                              )
                          for j, acc in enumerate(mm_extra):
                              nc.tensor.matmul(
                                  psB[:], id_t[:], acc[:, 20:25, :],
                                  start=False, stop=(j == len(mm_extra) - 1),
                              )
                          nc.scalar.activation(ft[:, 500:625], psB[:], copyfn)
                          feat[mc] = ft

                          if mc < NCH - 1:
                              continue

                          # head conv1 + BN/ReLU
                          for mq in range(NCH):
                              yt = apool.tile([128, HO * HO], bf16,
                                              name=f"y{mq}", tag=f"y{mq}",
                                              bufs=2)
                              for ph, (c0, c1) in enumerate(((0, 500),
                                                            (500, 625))):
                                  ps = ppool.tile([128, c1 - c0], f32,
                                                  name="ps_hd",
                                                  tag=("hdA" if ph == 0
                                                       else "hdB"),
                                                  bufs=(2 if ph == 0 else 1))
                                  for kc in range(NCH):
                                      nc.tensor.matmul(
                                          ps[:],
                                          wh1_t[kc][:, mq * 128:(mq + 1) * 128],
                                          feat[kc][:, c0:c1],
                                          start=(kc == 0), stop=(kc == NCH - 1),
                                      )
                                  nc.scalar.activation(yt[:, c0:c1], ps[:],
                                                       relu, bias=shh_t[mq][:],
                                                       scale=sch_t[mq][:])
                              ys[mq] = yt

                          # head conv2 + bias
                          ot = apool.tile([COUT, HO * HO], f32,
                                          name="ot", tag="ot", bufs=2)
                          for ph, (c0, c1) in enumerate(((0, 500), (500, 625))):
                              ps = ppool.tile([COUT, c1 - c0], f32,
                                              name="ps_o",
                                              tag=("hdA" if ph == 0 else "hdB"),
                                              bufs=(2 if ph == 0 else 1))
                              for kc in range(NCH):
                                  nc.tensor.matmul(
                                      ps[:],
                                      wh2_t[kc][:],
                                      ys[kc][:, c0:c1],
                                      start=(kc == 0), stop=(kc == NCH - 1),
                                  )
                              nc.scalar.activation(ot[:, c0:c1], ps[:], idfn,
                                                   bias=bh2_t[:], scale=1.0)
                          nc.sync.dma_start(
                              out[b_abs][:].rearrange("o h w -> o (h w)"), ot[:])
                  if s_feat_next is not None:
                      s_feat = s_feat_next

    nc.compile()
    return nc


def _get_nc():
    if "nc" not in _CACHE:
        _CACHE["nc"] = _build_nc()
    return _CACHE["nc"]


def kernel(kernel, search, w_k, g_k, b_k, m_k, v_k, w_s, g_s, b_s, m_s, v_s,
           w_h1, g_h, b_h, m_h, v_h, w_h2, bias_h2):
    from concourse.bass_utils import run_bass_kernel_spmd

    def fold(g, b, m, v):
        sc = (g / np.sqrt(v + EPS)).astype(np.float32)
        sh = (b - m * sc).astype(np.float32)
        return sc.reshape(-1, 1), sh.reshape(-1, 1)

    kernel, search, w_k, w_s, w_h1, w_h2, bias_h2 = [
        np.asarray(a) for a in
        (kernel, search, w_k, w_s, w_h1, w_h2, bias_h2)]
    g_k, b_k, m_k, v_k = map(np.asarray, (g_k, b_k, m_k, v_k))
    g_s, b_s, m_s, v_s = map(np.asarray, (g_s, b_s, m_s, v_s))
    g_h, b_h, m_h, v_h = map(np.asarray, (g_h, b_h, m_h, v_h))

    sck, shk = fold(g_k, b_k, m_k, v_k)
    scs, shs = fold(g_s, b_s, m_s, v_s)
    sch, shh = fold(g_h, b_h, m_h, v_h)

    common = {
        "wkT": np.ascontiguousarray(w_k.T).astype(BF16),
        "wsT": np.ascontiguousarray(w_s.T).astype(BF16),
        "wh1T": np.ascontiguousarray(w_h1.T).astype(BF16),
        "wh2T": np.ascontiguousarray(w_h2.T).astype(BF16),
        "sck": sck, "shk": shk, "scs": scs, "shs": shs,
        "sch": sch, "shh": shh,
        "bh2": bias_h2.astype(np.float32).reshape(-1, 1),
        "ident": np.eye(128, dtype=BF16),
    }
    xk_all = kernel.astype(BF16)
    xs_all = search.astype(BF16)

    in_maps = []
    for i in range(NCORES):
        bs = slice(i * NB, (i + 1) * NB)
        m = dict(common)
        m["xk"] = np.ascontiguousarray(xk_all[bs])
        m["xs"] = np.ascontiguousarray(xs_all[bs])
        in_maps.append(m)

    nc = _get_nc()
    res = run_bass_kernel_spmd(nc, in_maps, core_ids=list(range(NCORES)))
    return np.concatenate([res.results[i]["out"] for i in range(NCORES)],
                          axis=0)



# revision 45
# speedup vs baseline: 1.4097x; 1.3498x over previous
"""Trainium2 Bass kernel for DepthwiseXCorr (SiamRPN-style head).

Pipeline per batch sample:
  k = BN+ReLU(conv1x1(kernel, w_k))      [256, 7, 7]
  s = BN+ReLU(conv1x1(search, w_s))      [256, 31, 31]
  feat = depthwise_xcorr(s, k)           [256, 25, 25]
  y = BN+ReLU(conv1x1(feat, w_h1))
  out = conv1x1(y, w_h2) + bias          [20, 25, 25]

Sharding: pure data-parallel, batch 128 -> 16 per core across 8 cores.

Implementation notes:
- conv1x1 = matmul with channels on the contraction (partition) dim.
- depthwise xcorr: the 49 kernel taps (u, v) are split across engines.
  33 taps run on the PE as diag(k[:, u, v]) @ s[:, i+u, j+v] matmuls
  accumulating in PSUM (a depthwise tap can use at most 128 of the
  128x128 array's rows, so the PE alone would need ~410us); the diagonal
  weight tiles are built by scaling an identity matrix with a
  per-partition scalar (12 on VectorE at 4x bf16 mode, 21 on ScalarE,
  which has slack).  The other 16 taps run on VectorE as 4x-mode
  tensor_scalar mults plus 2x-mode tensor_tensor adds into a bf16
  accumulator that is folded into the PSUM group by one extra
  identity-weight matmul per phase.  Shifted search windows are plain
  strided APs into the search feature tile - no data movement.
- All matmuls in bf16 (PE full column rate), accumulation in fp32 PSUM,
  BN+ReLU epilogues on ScalarE with fp32 math, fp32 output.
- Cost-model span ~353 us; measured marginal HW time ~380-430 us
  (all three compute engines ~90% busy).
"""

import sys

if "/opt/trn_rl_repo" not in sys.path:
    sys.path.insert(0, "/opt/trn_rl_repo")

import ml_dtypes
import numpy as np

B, CIN, HID, COUT = 128, 256, 256, 20
NCORES = 8
NB = B // NCORES          # batches per core
HS = 31                   # search spatial
HK = 7                    # kernel spatial
HO = HS - HK + 1          # 25, xcorr output spatial
EPS = 1e-5
GB = 4                    # batch group size for the search-branch pipeline
NCH = 2                   # channel chunks of 128
P_TAPS = 33               # xcorr taps on PE (diag matmuls)
Q_TAPS = 0                # taps whose mult runs on ScalarE (add on VectorE)
D_TAPS = 49 - P_TAPS - Q_TAPS  # taps fully on VectorE (mult + add pairs)
DIAG_DVE = 11             # diag builds on DVE; the rest go to ScalarE
BF16 = ml_dtypes.bfloat16

_CACHE = {}


def _build_nc(repeat=1):
    import concourse.bacc as bacc
    import concourse.tile as tile
    from concourse import mybir

    f32 = mybir.dt.float32
    bf16 = mybir.dt.bfloat16

    nc = bacc.Bacc()

    xk = nc.dram_tensor("xk", [NB, CIN, HK, HK], bf16, kind="ExternalInput")
    xs = nc.dram_tensor("xs", [NB, CIN, HS, HS], bf16, kind="ExternalInput")
    wkT = nc.dram_tensor("wkT", [CIN, HID], bf16, kind="ExternalInput")
    wsT = nc.dram_tensor("wsT", [CIN, HID], bf16, kind="ExternalInput")
    wh1T = nc.dram_tensor("wh1T", [HID, HID], bf16, kind="ExternalInput")
    wh2T = nc.dram_tensor("wh2T", [HID, COUT], bf16, kind="ExternalInput")
    sck = nc.dram_tensor("sck", [HID, 1], f32, kind="ExternalInput")
    shk = nc.dram_tensor("shk", [HID, 1], f32, kind="ExternalInput")
    scs = nc.dram_tensor("scs", [HID, 1], f32, kind="ExternalInput")
    shs = nc.dram_tensor("shs", [HID, 1], f32, kind="ExternalInput")
    sch = nc.dram_tensor("sch", [HID, 1], f32, kind="ExternalInput")
    shh = nc.dram_tensor("shh", [HID, 1], f32, kind="ExternalInput")
    bh2 = nc.dram_tensor("bh2", [COUT, 1], f32, kind="ExternalInput")
    ident = nc.dram_tensor("ident", [128, 128], bf16, kind="ExternalInput")
    out = nc.dram_tensor("out", [NB, COUT, HO, HO], f32, kind="ExternalOutput")

    relu = mybir.ActivationFunctionType.Relu
    idfn = mybir.ActivationFunctionType.Identity
    copyfn = mybir.ActivationFunctionType.Copy

    with tile.TileContext(nc) as tc:
        with (
            tc.tile_pool(name="const", bufs=1) as cpool,
            tc.tile_pool(name="act", bufs=1) as apool,
            tc.tile_pool(name="stream", bufs=2) as spool,
            tc.tile_pool(name="diagp", bufs=3 * P_TAPS) as dpool,
            tc.tile_pool(name="psum", bufs=1, space="PSUM") as ppool,
        ):
            # ---- constants -------------------------------------------------
            # critical-path loads first: the kernel branch gates the xcorr
            # diag builds, so wk/sck/shk/ident/xk go ahead of everything.
            wk_t, ws_t, wh1_t, wh2_t = [], [], [], []
            sck_t, shk_t, scs_t, shs_t, sch_t, shh_t = [], [], [], [], [], []

            def _vec(vec_d, lst, nm, kc, sl):
                v = cpool.tile([128, 1], f32, name=f"{nm}_{kc}")
                nc.sync.dma_start(v[:], vec_d[sl, :])
                lst.append(v)

            for kc in range(NCH):
                sl = slice(kc * 128, (kc + 1) * 128)
                w1 = cpool.tile([128, HID], bf16, name=f"wk_{kc}")
                nc.sync.dma_start(w1[:], wkT[sl, :])
                wk_t.append(w1)
                _vec(sck, sck_t, "sck", kc, sl)
                _vec(shk, shk_t, "shk", kc, sl)
            id_t = cpool.tile([128, 128], bf16, name="id_t")
            nc.sync.dma_start(id_t[:], ident[:])
            xk_ap = xk[:].rearrange("b c h w -> c b (h w)")
            xk_t = []
            for kc in range(NCH):
                t = apool.tile([128, NB, HK * HK], bf16, name=f"xk_t{kc}")
                nc.gpsimd.dma_start(t[:], xk_ap[kc * 128:(kc + 1) * 128])
                xk_t.append(t)
            for kc in range(NCH):
                sl = slice(kc * 128, (kc + 1) * 128)
                w2 = cpool.tile([128, HID], bf16, name=f"ws_{kc}")
                nc.sync.dma_start(w2[:], wsT[sl, :])
                ws_t.append(w2)
                _vec(scs, scs_t, "scs", kc, sl)
                _vec(shs, shs_t, "shs", kc, sl)
            for kc in range(NCH):
                sl = slice(kc * 128, (kc + 1) * 128)
                w3 = cpool.tile([128, HID], bf16, name=f"wh1_{kc}")
                nc.sync.dma_start(w3[:], wh1T[sl, :])
                wh1_t.append(w3)
                w4 = cpool.tile([128, COUT], bf16, name=f"wh2_{kc}")
                nc.sync.dma_start(w4[:], wh2T[sl, :])
                wh2_t.append(w4)
                _vec(sch, sch_t, "sch", kc, sl)
                _vec(shh, shh_t, "shh", kc, sl)
            bh2_t = cpool.tile([COUT, 1], f32, name="bh2_t")
            nc.sync.dma_start(bh2_t[:], bh2[:])

            # ---- kernel branch conv (all NB batches at once) ---------------
            # k_feat[mc]: [128, NB, 49] fp32 (source of per-partition scalars)
            k_feat = []
            for mc in range(NCH):
                kf = apool.tile([128, NB, HK * HK], f32, name=f"k_feat{mc}")
                for half in range(2):
                    bs = slice(half * (NB // 2), (half + 1) * (NB // 2))
                    ps = ppool.tile([128, NB // 2, HK * HK], f32,
                                    name="ps_cs", tag="csA", bufs=1)
                    for kc in range(NCH):
                        nc.tensor.matmul(
                            ps[:],
                            wk_t[kc][:, mc * 128:(mc + 1) * 128],
                            xk_t[kc][:, bs, :],
                            start=(kc == 0), stop=(kc == NCH - 1),
                        )
                    nc.scalar.activation(kf[:, bs, :], ps[:], relu,
                                         bias=shk_t[mc][:], scale=sck_t[mc][:])
                k_feat.append(kf)

            for _rep in range(repeat):
              # ---- main loop over batch groups -------------------------------
              # xcorr iterations are (bl, mc) pairs
              NGRP = NB // GB
              iters = [(g, bl, mc) for g in range(NGRP)
                       for bl in range(GB) for mc in range(NCH)]
              pe_taps = {it: P_TAPS for it in iters}

              def emit_diags_dve(it):
                  g, bl, mc = it
                  b_abs = g * GB + bl
                  lst = []
                  for uv in range(DIAG_DVE):
                      dg = dpool.tile([128, 128], bf16, name="dg", tag="diag")
                      nc.vector.tensor_scalar_mul(
                          dg[:], id_t[:], k_feat[mc][:, b_abs, uv:uv + 1])
                      lst.append(dg)
                  return lst

              def emit_diags_act(it):
                  g, bl, mc = it
                  b_abs = g * GB + bl
                  lst = []
                  for uv in range(DIAG_DVE, pe_taps[it]):
                      dg = dpool.tile([128, 128], bf16, name="dg", tag="diag")
                      nc.scalar.mul(dg[:], id_t[:],
                                    k_feat[mc][:, b_abs, uv:uv + 1])
                      lst.append(dg)
                  return lst

              xs_ap = xs[:].rearrange("b c h w -> c b (h w)")
              feat = {}
              ys = {}

              def emit_group_conv(g):
                  """xs load + search conv + BN/ReLU for one batch group."""
                  gsl = slice(g * GB, (g + 1) * GB)
                  xs_t = []
                  for kc in range(NCH):
                      t = spool.tile([128, GB, HS * HS], bf16,
                                     name=f"xs_t{kc}", tag=f"xs{kc}")
                      nc.gpsimd.dma_start(t[:],
                                          xs_ap[kc * 128:(kc + 1) * 128, gsl])
                      xs_t.append(t)
                  s_feat = []
                  for mc in range(NCH):
                      sf = spool.tile([128, GB, HS, HS], bf16,
                                      name=f"s_feat{mc}", tag=f"sf{mc}")
                      for bl in range(GB):
                          for ph, (r0, r1) in enumerate(((0, 16), (16, 31))):
                              ps = ppool.tile([128, r1 - r0, HS], f32,
                                              name="ps_cs2",
                                              tag=("csA" if ph == 0 else "csB"),
                                              bufs=1)
                              for kc in range(NCH):
                                  nc.tensor.matmul(
                                      ps[:],
                                      ws_t[kc][:, mc * 128:(mc + 1) * 128],
                                      xs_t[kc][:, bl, r0 * HS:r1 * HS],
                                      start=(kc == 0), stop=(kc == NCH - 1),
                                  )
                              nc.scalar.activation(sf[:, bl, r0:r1, :], ps[:],
                                                   relu, bias=shs_t[mc][:],
                                                   scale=scs_t[mc][:])
                      s_feat.append(sf)
                  return s_feat

              s_feat = emit_group_conv(0)
              for g in range(NGRP):
                  s_feat_next = None
                  for bl in range(GB):
                      if bl == GB - 2 and g + 1 < NGRP:
                          # hoist the next group's conv ahead of this
                          # group's last batch so ScalarE/DVE are not
                          # starved at the group boundary
                          s_feat_next = emit_group_conv(g + 1)
                      b_abs = g * GB + bl
                      for mc in range(NCH):
                          it = (g, bl, mc)
                          kcol = lambda uv: k_feat[mc][:, b_abs, uv:uv + 1]
                          swin = lambda uv, r0, r1: s_feat[mc][
                              :, bl, uv // 7 + r0:uv // 7 + r1,
                              uv % 7:uv % 7 + HO]
                          diags = emit_diags_dve(it) + emit_diags_act(it)
                          # DVE taps -> bf16 accumulator (4x-mode mult into a
                          # temp + 2x-mode add; cheaper than the 1x fused STT)
                          n_pe = pe_taps[it]
                          n_dve = 49 - n_pe
                          mm_extra = []
                          if n_dve:
                              accd = apool.tile([128, HO, HO], bf16,
                                                name="accd", tag=f"accd{mc}",
                                                bufs=4)
                              for i in range(n_dve):
                                  uv = n_pe + i
                                  if i == 0:
                                      nc.vector.tensor_scalar_mul(
                                          accd[:], swin(uv, 0, HO), kcol(uv))
                                  else:
                                      tmp = apool.tile([128, HO, HO], bf16,
                                                       name="tmp", tag="tmp",
                                                       bufs=6)
                                      if i > n_dve - 1 - Q_TAPS:
                                          nc.scalar.mul(
                                              tmp[:], swin(uv, 0, HO),
                                              kcol(uv))
                                      else:
                                          nc.vector.tensor_scalar_mul(
                                              tmp[:], swin(uv, 0, HO),
                                              kcol(uv))
                                      nc.vector.tensor_tensor(
                                          accd[:], accd[:], tmp[:],
                                          mybir.AluOpType.add)
                              mm_extra.append(accd)
                          ft = apool.tile([128, HO * HO], bf16,
                                          name=f"feat{mc}", tag=f"f{mc}", bufs=3)
                          # phase A: output rows 0:20 (N=500)
                          psA = ppool.tile([128, 20 * HO], f32,
                                           name="ps_xc", tag="xc", bufs=3)
                          for uv in range(n_pe):
                              nc.tensor.matmul(
                                  psA[:], diags[uv][:], swin(uv, 0, 20),
                                  start=(uv == 0), stop=False,
                              )
                          for j, acc in enumerate(mm_extra):
                              nc.tensor.matmul(
                                  psA[:], id_t[:], acc[:, 0:20, :],
                                  start=False, stop=(j == len(mm_extra) - 1),
                              )
                          nc.scalar.activation(ft[:, 0:500], psA[:], copyfn)
                          # phase B: output rows 20:25 (N=125)
                          psB = ppool.tile([128, 5 * HO], f32,
                                           name="ps_xcB", tag="xc", bufs=3)
                          for uv in range(n_pe):
                              nc.tensor.matmul(
                                  psB[:], diags[uv][:], swin(uv, 20, 25),
                                  start=(uv == 0), stop=False,
                              )
                          for j, acc in enumerate(mm_extra):
                              nc.tensor.matmul(
                                  psB[:], id_t[:], acc[:, 20:25, :],
                                  start=False, stop=(j == len(mm_extra) - 1),
                              )
                          nc.scalar.activation(ft[:, 500:625], psB[:], copyfn)
                          feat[mc] = ft

                          if mc < NCH - 1:
                              continue

                          # head conv1 + BN/ReLU
                          for mq in range(NCH):
                              yt = apool.tile([128, HO * HO], bf16,
                                              name=f"y{mq}", tag=f"y{mq}",
                                              bufs=2)
                              for ph, (c0, c1) in enumerate(((0, 500),
                                                            (500, 625))):
                                  ps = ppool.tile([128, c1 - c0], f32,
                                                  name="ps_hd",
                                                  tag=("hdA" if ph == 0
                                                       else "hdB"),
                                                  bufs=(2 if ph == 0 else 1))
                                  for kc in range(NCH):
                                      nc.tensor.matmul(
                                          ps[:],
                                          wh1_t[kc][:, mq * 128:(mq + 1) * 128],
                                          feat[kc][:, c0:c1],
                                          start=(kc == 0), stop=(kc == NCH - 1),
                                      )
                                  nc.scalar.activation(yt[:, c0:c1], ps[:],
                                                       relu, bias=shh_t[mq][:],
                                                       scale=sch_t[mq][:])
                              ys[mq] = yt

                          # head conv2 + bias
                          ot = apool.tile([COUT, HO * HO], f32,
                                          name="ot", tag="ot", bufs=2)
                          for ph, (c0, c1) in enumerate(((0, 500), (500, 625))):
                              ps = ppool.tile([COUT, c1 - c0], f32,
                                              name="ps_o",
                                              tag=("hdA" if ph == 0 else "hdB"),
                                              bufs=(2 if ph == 0 else 1))
                              for kc in range(NCH):
                                  nc.tensor.matmul(
                                      ps[:],
                                      wh2_t[kc][:],
                                      ys[kc][:, c0:c1],
                                      start=(kc == 0), stop=(kc == NCH - 1),
                                  )
                              nc.scalar.activation(ot[:, c0:c1], ps[:], idfn,
                                                   bias=bh2_t[:], scale=1.0)
                          nc.sync.dma_start(
                              out[b_abs][:].rearrange("o h w -> o (h w)"), ot[:])
                  if s_feat_next is not None:
                      s_feat = s_feat_next

    nc.compile()
    return nc


def _get_nc():
    if "nc" not in _CACHE:
        _CACHE["nc"] = _build_nc()
    return _CACHE["nc"]


def kernel(kernel, search, w_k, g_k, b_k, m_k, v_k, w_s, g_s, b_s, m_s, v_s,
           w_h1, g_h, b_h, m_h, v_h, w_h2, bias_h2):
    from concourse.bass_utils import run_bass_kernel_spmd

    def fold(g, b, m, v):
        sc = (g / np.sqrt(v + EPS)).astype(np.float32)
        sh = (b - m * sc).astype(np.float32)
        return sc.reshape(-1, 1), sh.reshape(-1, 1)

    kernel, search, w_k, w_s, w_h1, w_h2, bias_h2 = [
        np.asarray(a) for a in
        (kernel, search, w_k, w_s, w_h1, w_h2, bias_h2)]
    g_k, b_k, m_k, v_k = map(np.asarray, (g_k, b_k, m_k, v_k))
    g_s, b_s, m_s, v_s = map(np.asarray, (g_s, b_s, m_s, v_s))
    g_h, b_h, m_h, v_h = map(np.asarray, (g_h, b_h, m_h, v_h))

    sck, shk = fold(g_k, b_k, m_k, v_k)
    scs, shs = fold(g_s, b_s, m_s, v_s)
    sch, shh = fold(g_h, b_h, m_h, v_h)

    common = {
        "wkT": np.ascontiguousarray(w_k.T).astype(BF16),
        "wsT": np.ascontiguousarray(w_s.T).astype(BF16),
        "wh1T": np.ascontiguousarray(w_h1.T).astype(BF16),
        "wh2T": np.ascontiguousarray(w_h2.T).astype(BF16),
        "sck": sck, "shk": shk, "scs": scs, "shs": shs,
        "sch": sch, "shh": shh,
        "bh2": bias_h2.astype(np.float32).reshape(-1, 1),
        "ident": np.eye(128, dtype=BF16),
    }
    xk_all = kernel.astype(BF16)
    xs_all = search.astype(BF16)

    in_maps = []
    for i in range(NCORES):
        bs = slice(i * NB, (i + 1) * NB)
        m = dict(common)
        m["xk"] = np.ascontiguousarray(xk_all[bs])
        m["xs"] = np.ascontiguousarray(xs_all[bs])
        in_maps.append(m)

    nc = _get_nc()
    res = run_bass_kernel_spmd(nc, in_maps, core_ids=list(range(NCORES)))
    return np.concatenate([res.results[i]["out"] for i in range(NCORES)],
                          axis=0)

